# revision 8
# baseline (speedup 1.0000x reference)
"""GNN message-passing kernel for Trainium2 (8 NeuronCores, edge-parallel).

Strategy: shard edges by source-node range. Each core packs its edges into
"groups" of <=128 distinct src nodes and <=768 edges (6 chunks of 128).
Each group owns 128 output slots; host maps slots back to node ids.
Outputs are disjoint -> no collective; host concatenates + adds the
edge-type-embedding/bias terms (computed host-side from counts).

Device pipeline per chunk (128 edges):
  PE : Q/K/V projections with bias pre-loaded into PSUM via a ones-row
       matmul (start=True), so no bias-add is needed on DVE.
  ACT: PSUM->SBUF bf16 eviction copies; exp(scores) per group.
  DVE: 8x8x32 attention via broadcast-AP multiply + log-tree adds
       (tensor_reduce runs at 1 elem/cycle; TT-adds run at 2/cycle).
  PE : segment-sum via one-hot matmul (S from host), final Wo projection.
"""

import sys

sys.path.insert(0, "/opt/trn_rl_repo")

import numpy as np
import ml_dtypes

from concourse import bass, bacc, mybir
import concourse.tile as tile
from concourse.bass_utils import run_bass_kernel_spmd

N_NODES = 50000
N_CORES = 8
NPC = N_NODES // N_CORES  # 6250
IN_DIM = 128
HID = 256
H = 8
D = 32
CPG = 6                   # chunks per group
EPG = CPG * 128           # 768 edges per group

BF16 = ml_dtypes.bfloat16
_prog_cache = {}


def _build_program(G):
    T = G * CPG
    E_pad = T * 128
    f32, bf16 = mybir.dt.float32, mybir.dt.bfloat16
    X = mybir.AxisListType.X
    MUL, ADD = mybir.AluOpType.mult, mybir.AluOpType.add
    Copy = mybir.ActivationFunctionType.Copy
    Exp = mybir.ActivationFunctionType.Exp

    nc = bacc.Bacc("TRN2", target_bir_lowering=False)
    xsT = nc.dram_tensor("xsT", [128, E_pad], bf16, kind="ExternalInput")
    xtT = nc.dram_tensor("xtT", [128, E_pad], bf16, kind="ExternalInput")
    S2 = nc.dram_tensor("S2", [128, E_pad], bf16, kind="ExternalInput")
    Wt = nc.dram_tensor("Wt", [128, 768], bf16, kind="ExternalInput")
    Brow = nc.dram_tensor("Brow", [1, 768], bf16, kind="ExternalInput")
    Ones = nc.dram_tensor("Ones", [1, 128], bf16, kind="ExternalInput")
    W2 = nc.dram_tensor("W2", [128, 256], bf16, kind="ExternalInput")
    out = nc.dram_tensor("out", [128, G * 128], f32, kind="ExternalOutput")

    with tile.TileContext(nc) as tc:
        with tc.tile_pool(name="const", bufs=1) as cp, \
             tc.tile_pool(name="io", bufs=2) as iop, \
             tc.tile_pool(name="work", bufs=2) as wp, \
             tc.tile_pool(name="pproj", bufs=2, space="PSUM") as pp, \
             tc.tile_pool(name="pacc", bufs=1, space="PSUM") as pa:

            wt = cp.tile([128, 768], bf16)
            nc.sync.dma_start(out=wt[:], in_=Wt[:, :])
            brow = cp.tile([1, 768], bf16)
            nc.sync.dma_start(out=brow[:], in_=Brow[:, :])
            ones = cp.tile([1, 128], bf16)
            nc.sync.dma_start(out=ones[:], in_=Ones[:, :])
            w2 = cp.tile([128, 256], bf16)
            nc.sync.dma_start(out=w2[:], in_=W2[:, :])
            outsb = cp.tile([128, G * 128], f32)

            def proj_group(g):
                """DMA + QKV projections + evictions for group g. Emitted a
                group ahead so ACT's eviction copies are queued before the
                previous group's exp and never stall the DVE."""
                esl = slice(g * EPG, (g + 1) * EPG)
                xs = iop.tile([128, EPG], bf16, tag="xs")
                nc.sync.dma_start(out=xs[:], in_=xsT[:, esl])
                xt = iop.tile([128, EPG], bf16, tag="xt")
                nc.sync.dma_start(out=xt[:], in_=xtT[:, esl])
                sg = iop.tile([128, EPG], bf16, tag="sg")
                nc.sync.dma_start(out=sg[:], in_=S2[:, esl])
                qsb = wp.tile([128, CPG * 256], bf16, tag="qsb")
                kvsb = wp.tile([128, CPG * 512], bf16, tag="kvsb")
                for i in range(CPG):
                    ei = slice(i * 128, (i + 1) * 128)
                    ps_q = pp.tile([128, 256], f32, tag="psq")
                    ps_kv = pp.tile([128, 512], f32, tag="pskv")
                    # bias rows pre-loaded via ones-row matmul, then the
                    # projection accumulates on top (start=False)
                    nc.tensor.matmul(ps_q[:], lhsT=ones[:, :],
                                     rhs=brow[:, 0:256], start=True, stop=False)
                    nc.tensor.matmul(ps_kv[:], lhsT=ones[:, :],
                                     rhs=brow[:, 256:768], start=True, stop=False)
                    nc.tensor.matmul(ps_q[:], lhsT=xs[:, ei],
                                     rhs=wt[:, 0:256], start=False, stop=True)
                    nc.tensor.matmul(ps_kv[:], lhsT=xt[:, ei],
                                     rhs=wt[:, 256:768], start=False, stop=True)
                    nc.scalar.activation(out=qsb[:, i * 256:(i + 1) * 256],
                                         in_=ps_q[:], func=Copy)
                    nc.scalar.activation(out=kvsb[:, i * 512:(i + 1) * 512],
                                         in_=ps_kv[:], func=Copy)
                return xs, xt, sg, qsb, kvsb

            def dve_group(g, tiles):
                xs, xt, sg, qsb, kvsb = tiles
                sgrp = wp.tile([128, CPG * 64], f32, tag="sgrp")
                prodg = wp.tile([128, CPG * 2048], bf16, tag="prod")
                for i in range(CPG):
                    ci = slice(i * 256, (i + 1) * 256)
                    # prod[e, h, g, d] = Q[e,h,d] * K[e,g,d]
                    qa = (qsb[:, ci]
                          .rearrange("p (h d) -> p h d", h=H)
                          .unsqueeze(2).to_broadcast([128, H, H, D]))
                    ka = (kvsb[:, i * 512:i * 512 + 256]
                          .rearrange("p (g d) -> p g d", g=H)
                          .unsqueeze(1).to_broadcast([128, H, H, D]))
                    nc.vector.tensor_tensor(
                        out=prodg[:, i * 2048:(i + 1) * 2048]
                            .rearrange("p (h g d) -> p h g d", h=H, g=H),
                        in0=qa, in1=ka, op=MUL)

                # group-wide log-tree reduce over d (TT-adds run 2/cycle vs
                # 1 for tensor_reduce; one instruction per level for all 6
                # chunks amortizes the SBUF access bubble)
                A = CPG * 64
                pr = prodg[:].rearrange("p (a d) -> p a d", d=32)
                t1 = wp.tile([128, A * 16], bf16, tag="t1")
                t1r = t1[:].rearrange("p (a d) -> p a d", d=16)
                nc.vector.tensor_tensor(out=t1r, in0=pr[:, :, 0:16],
                                        in1=pr[:, :, 16:32], op=ADD)
                t2 = wp.tile([128, A * 8], bf16, tag="t2")
                t2r = t2[:].rearrange("p (a d) -> p a d", d=8)
                nc.vector.tensor_tensor(out=t2r, in0=t1r[:, :, 0:8],
                                        in1=t1r[:, :, 8:16], op=ADD)
                t3 = wp.tile([128, A * 4], bf16, tag="t3")
                t3r = t3[:].rearrange("p (a d) -> p a d", d=4)
                nc.vector.tensor_tensor(out=t3r, in0=t2r[:, :, 0:4],
                                        in1=t2r[:, :, 4:8], op=ADD)
                t4 = wp.tile([128, A * 2], bf16, tag="t4")
                t4r = t4[:].rearrange("p (a d) -> p a d", d=2)
                nc.vector.tensor_tensor(out=t4r, in0=t3r[:, :, 0:2],
                                        in1=t3r[:, :, 2:4], op=ADD)
                nc.vector.tensor_tensor(
                    out=sgrp[:].rearrange("p (a o) -> p a o", o=1),
                    in0=t4r[:, :, 0:1], in1=t4r[:, :, 1:2], op=ADD)

                # softmax over g for the whole group
                u = wp.tile([128, CPG * 64], f32, tag="u")
                nc.scalar.activation(out=u[:], in_=sgrp[:], func=Exp,
                                     scale=float(1.0 / np.sqrt(D)))
                zs = wp.tile([128, CPG * 8], f32, tag="zs")
                nc.vector.tensor_reduce(
                    out=zs[:], in_=u[:].rearrange("p (a g) -> p a g", g=H),
                    axis=X, op=ADD)
                rinv = wp.tile([128, CPG * 8], f32, tag="rinv")
                nc.vector.reciprocal(out=rinv[:], in_=zs[:])
                attn = wp.tile([128, CPG * 64], bf16, tag="attn")
                nc.vector.tensor_tensor(
                    out=attn[:].rearrange("p (a g) -> p a g", g=H),
                    in0=u[:].rearrange("p (a g) -> p a g", g=H),
                    in1=rinv[:].rearrange("p (a o) -> p a o", o=1)
                        .to_broadcast([128, CPG * 8, H]),
                    op=MUL)

                msgsb = wp.tile([128, CPG * 256], bf16, tag="msg")
                prod2g = wp.tile([128, CPG * 2048], bf16, tag="prod")
                for i in range(CPG):
                    # msg[e, h, d] = sum_g attn[e,h,g] * V[e,d,g] (V host-permuted)
                    aa = (attn[:, i * 64:(i + 1) * 64]
                          .rearrange("p (h g) -> p h g", h=H)
                          .unsqueeze(2).to_broadcast([128, H, D, H]))
                    va = (kvsb[:, i * 512 + 256:(i + 1) * 512]
                          .rearrange("p (d g) -> p d g", d=D)
                          .unsqueeze(1).to_broadcast([128, H, D, H]))
                    nc.vector.tensor_tensor(
                        out=prod2g[:, i * 2048:(i + 1) * 2048]
                            .rearrange("p (h d g) -> p h d g", h=H, d=D),
                        in0=aa, in1=va, op=MUL)
                # group-wide tree over g
                B = CPG * 256
                p2 = prod2g[:].rearrange("p (a g) -> p a g", g=8)
                r1 = wp.tile([128, B * 4], bf16, tag="t1")
                r1r = r1[:].rearrange("p (a g) -> p a g", g=4)
                nc.vector.tensor_tensor(out=r1r, in0=p2[:, :, 0:4],
                                        in1=p2[:, :, 4:8], op=ADD)
                r2 = wp.tile([128, B * 2], bf16, tag="t2")
                r2r = r2[:].rearrange("p (a g) -> p a g", g=2)
                nc.vector.tensor_tensor(out=r2r, in0=r1r[:, :, 0:2],
                                        in1=r1r[:, :, 2:4], op=ADD)
                nc.vector.tensor_tensor(
                    out=msgsb[:].rearrange("p (a o) -> p a o", o=1),
                    in0=r2r[:, :, 0:1], in1=r2r[:, :, 1:2], op=ADD)

                agg1 = pa.tile([128, 128], f32, tag="agg1")
                agg2 = pa.tile([128, 128], f32, tag="agg2")
                for i in range(CPG):
                    ei = slice(i * 128, (i + 1) * 128)
                    st, sp = (i == 0), (i == CPG - 1)
                    nc.tensor.matmul(agg1[:],
                                     lhsT=msgsb[:, i * 256:i * 256 + 128],
                                     rhs=sg[:, ei], start=st, stop=sp)
                    nc.tensor.matmul(agg2[:],
                                     lhsT=msgsb[:, i * 256 + 128:(i + 1) * 256],
                                     rhs=sg[:, ei], start=st, stop=sp)

                a1 = wp.tile([128, 128], bf16, tag="a1")
                nc.scalar.activation(out=a1[:], in_=agg1[:], func=Copy)
                a2 = wp.tile([128, 128], bf16, tag="a2")
                nc.scalar.activation(out=a2[:], in_=agg2[:], func=Copy)
                mt = pa.tile([128, 128], f32, tag="mt")
                nc.tensor.matmul(mt[:], lhsT=w2[:, 0:128], rhs=a1[:],
                                 start=True, stop=False)
                nc.tensor.matmul(mt[:], lhsT=w2[:, 128:256], rhs=a2[:],
                                 start=False, stop=True)
                nc.scalar.activation(out=outsb[:, g * 128:(g + 1) * 128],
                                     in_=mt[:], func=Copy)

            pending = proj_group(0)
            for g in range(G):
                nxt = proj_group(g + 1) if g + 1 < G else None
                dve_group(g, pending)
                pending = nxt

            nc.sync.dma_start(out=out[:, :], in_=outsb[:])
    return nc


def _pack_cores(x, src, tgt, et):
    """Group-pack each core's edges: <=128 distinct src nodes and <=EPG
    edges per group. Returns per-core packing plus global G."""
    cores = []
    Gmax = 1
    for c in range(N_CORES):
        idx = np.nonzero(src // NPC == c)[0]
        s_loc = src[idx] - c * NPC
        order = np.argsort(s_loc, kind="stable")
        idx, s_loc = idx[order], s_loc[order]
        deg = np.bincount(s_loc, minlength=NPC)
        nz = np.nonzero(deg)[0]
        # sequential node-granularity packing
        node_grp = np.empty(len(nz), np.int64)
        node_slot = np.empty(len(nz), np.int64)
        g = 0; nslots = 0; nedges = 0
        for j, n in enumerate(nz):
            d = deg[n]
            if nslots >= 128 or nedges + d > EPG:
                g += 1; nslots = 0; nedges = 0
            node_grp[j] = g
            node_slot[j] = nslots
            nslots += 1
            nedges += d
        G_core = g + 1
        Gmax = max(Gmax, G_core)
        cores.append((idx, s_loc, deg, nz, node_grp, node_slot, G_core))
    return cores, Gmax


def prepare(node_features, edges, edge_types, Wq, bq, Wk, bk, Wv, bv,
            edge_emb, Wo, bo):
    """Host-side sharding/pack. Returns (nc, in_maps, meta)."""
    x = np.asarray(node_features, dtype=np.float32)
    edges = np.asarray(edges, dtype=np.int64)
    et = np.asarray(edge_types, dtype=np.int64)
    Wq = np.asarray(Wq, np.float32); bq = np.asarray(bq, np.float32)
    Wk = np.asarray(Wk, np.float32); bk = np.asarray(bk, np.float32)
    Wv = np.asarray(Wv, np.float32); bv = np.asarray(bv, np.float32)
    edge_emb = np.asarray(edge_emb, np.float32)
    Wo = np.asarray(Wo, np.float32); bo = np.asarray(bo, np.float32)

    src, tgt = edges[:, 0], edges[:, 1]
    cores, G = _pack_cores(x, src, tgt, et)
    E_pad = G * EPG

    # host-side: edge-type embedding + output bias terms (linear in counts)
    M = edge_emb @ Wo.T                                  # [3, 128]
    cnt = np.zeros((N_NODES, edge_emb.shape[0]), np.float32)
    np.add.at(cnt, (src, et), 1.0)
    deg_all = np.bincount(src, minlength=N_NODES).astype(np.float32)
    cnt_extra = cnt @ M + deg_all[:, None] * bo[None, :]  # [N, 128]

    # shared weight blocks
    Wvp = Wv.reshape(H, D, IN_DIM).transpose(1, 0, 2).reshape(HID, IN_DIM)
    bvp = bv.reshape(H, D).T.reshape(HID)
    Wt_np = np.concatenate([Wq.T, Wk.T, Wvp.T], axis=1).astype(BF16)
    Brow_np = np.concatenate([bq, bk, bvp])[None, :].astype(BF16)
    Ones_np = np.ones((1, 128), BF16)
    WoT = Wo.T.astype(np.float32)
    W2_np = np.concatenate([WoT[0:128], WoT[128:256]], axis=1).astype(BF16)

    in_maps = []
    slot_nodes = []
    for c in range(N_CORES):
        idx, s_loc, deg, nz, node_grp, node_slot, G_core = cores[c]
        # per-node group/slot -> per-edge positions (edges are sorted by
        # s_loc so each node's edges are consecutive)
        grp_of_node = np.zeros(NPC, np.int64)
        slot_of_node = np.zeros(NPC, np.int64)
        grp_of_node[nz] = node_grp
        slot_of_node[nz] = node_slot
        # edge offset within its group: running count per group
        e_grp = grp_of_node[s_loc]
        # within-group position: stable running index per group value
        grp_counts = np.bincount(e_grp, minlength=G)
        grp_starts = np.zeros(G, np.int64)
        grp_starts[1:] = np.cumsum(grp_counts)[:-1]
        # edges sorted by s_loc are also sorted by group (groups ascend)
        within = np.arange(len(idx)) - grp_starts[e_grp]
        pos = e_grp * EPG + within

        xs_full = np.zeros((E_pad, IN_DIM), np.float32)
        xs_full[pos] = x[src[idx]]
        xt_full = np.zeros((E_pad, IN_DIM), np.float32)
        xt_full[pos] = x[tgt[idx]]
        S_full = np.zeros((E_pad, 128), np.float32)
        S_full[pos, slot_of_node[s_loc]] = 1.0

        T = G * CPG
        in_maps.append({
            "xsT": np.ascontiguousarray(xs_full.T).astype(BF16),
            "xtT": np.ascontiguousarray(xt_full.T).astype(BF16),
            "S2": np.ascontiguousarray(
                S_full.reshape(T, 128, 128).transpose(1, 0, 2)
                .reshape(128, T * 128)).astype(BF16),
            "Wt": Wt_np, "Brow": Brow_np, "Ones": Ones_np, "W2": W2_np,
        })
        node_of_slot = np.full(G * 128, -1, np.int64)
        node_of_slot[node_grp * 128 + node_slot] = nz + c * NPC
        slot_nodes.append(node_of_slot)

    if G not in _prog_cache:
        nc = _build_program(G)
        nc.finalize()
        _prog_cache[G] = nc
    nc = _prog_cache[G]
    meta = {"slot_nodes": slot_nodes, "cnt_extra": cnt_extra, "G": G}
    return nc, in_maps, meta


def unpack(outs, meta):
    messages = meta["cnt_extra"].astype(np.float32).copy()
    for c in range(N_CORES):
        o = np.asarray(outs[c]["out"], dtype=np.float32)  # [128, G*128]
        nos = meta["slot_nodes"][c]
        valid = nos >= 0
        messages[nos[valid]] += o[:, valid].T
    return messages


def kernel(node_features, edges, edge_types, Wq, bq, Wk, bk, Wv, bv,
           edge_emb, Wo, bo):
    nc, in_maps, meta = prepare(node_features, edges, edge_types, Wq, bq,
                                Wk, bk, Wv, bv, edge_emb, Wo, bo)
    res = run_bass_kernel_spmd(nc, in_maps, core_ids=list(range(N_CORES)))
    return unpack(res.results, meta)


# revision 10
# speedup vs baseline: 1.0567x; 1.0567x over previous
"""GNN message-passing kernel for Trainium2 (8 NeuronCores, edge-parallel).

Strategy: shard edges by source-node range. Each core packs its edges into
"groups" of <=128 distinct src nodes and <=768 edges (6 chunks of 128).
Each group owns 128 output slots; host maps slots back to node ids.
Outputs are disjoint -> no collective; host concatenates + adds the
edge-type-embedding/bias terms (computed host-side from counts).

Device pipeline per chunk (128 edges):
  PE : Q/K/V projections with bias pre-loaded into PSUM via a ones-row
       matmul (start=True), so no bias-add is needed on DVE.
  ACT: PSUM->SBUF bf16 eviction copies; exp(scores) per group.
  DVE: 8x8x32 attention via broadcast-AP multiply + log-tree adds
       (tensor_reduce runs at 1 elem/cycle; TT-adds run at 2/cycle).
  PE : segment-sum via one-hot matmul (S from host), final Wo projection.
"""

import sys

sys.path.insert(0, "/opt/trn_rl_repo")

import numpy as np
import ml_dtypes

from concourse import bass, bacc, mybir
import concourse.tile as tile
from concourse.bass_utils import run_bass_kernel_spmd

N_NODES = 50000
N_CORES = 8
NPC = N_NODES // N_CORES  # 6250
IN_DIM = 128
HID = 256
H = 8
D = 32
CPG = 6                   # chunks per group
EPG = CPG * 128           # 768 edges per group

BF16 = ml_dtypes.bfloat16
_prog_cache = {}


def _build_program(G):
    T = G * CPG
    E_pad = T * 128
    f32, bf16 = mybir.dt.float32, mybir.dt.bfloat16
    X = mybir.AxisListType.X
    MUL, ADD = mybir.AluOpType.mult, mybir.AluOpType.add
    Copy = mybir.ActivationFunctionType.Copy
    Exp = mybir.ActivationFunctionType.Exp

    nc = bacc.Bacc("TRN2", target_bir_lowering=False)
    xsT = nc.dram_tensor("xsT", [128, E_pad], bf16, kind="ExternalInput")
    xtT = nc.dram_tensor("xtT", [128, E_pad], bf16, kind="ExternalInput")
    S2 = nc.dram_tensor("S2", [128, E_pad], bf16, kind="ExternalInput")
    Wt = nc.dram_tensor("Wt", [128, 768], bf16, kind="ExternalInput")
    Brow = nc.dram_tensor("Brow", [1, 768], bf16, kind="ExternalInput")
    Ones = nc.dram_tensor("Ones", [1, 128], bf16, kind="ExternalInput")
    W2 = nc.dram_tensor("W2", [128, 256], bf16, kind="ExternalInput")
    out = nc.dram_tensor("out", [128, G * 128], f32, kind="ExternalOutput")

    with tile.TileContext(nc) as tc:
        with tc.tile_pool(name="const", bufs=1) as cp, \
             tc.tile_pool(name="io", bufs=2) as iop, \
             tc.tile_pool(name="work", bufs=2) as wp, \
             tc.tile_pool(name="pproj", bufs=2, space="PSUM") as pp, \
             tc.tile_pool(name="pacc", bufs=1, space="PSUM") as pa:

            wt = cp.tile([128, 768], bf16)
            nc.sync.dma_start(out=wt[:], in_=Wt[:, :])
            brow = cp.tile([1, 768], bf16)
            nc.sync.dma_start(out=brow[:], in_=Brow[:, :])
            ones = cp.tile([1, 128], bf16)
            nc.sync.dma_start(out=ones[:], in_=Ones[:, :])
            w2 = cp.tile([128, 256], bf16)
            nc.sync.dma_start(out=w2[:], in_=W2[:, :])
            outsb = cp.tile([128, G * 128], f32)

            def proj_group(g):
                """DMA + QKV projections + evictions for group g. Emitted a
                group ahead so ACT's eviction copies are queued before the
                previous group's exp and never stall the DVE."""
                esl = slice(g * EPG, (g + 1) * EPG)
                xs = iop.tile([128, EPG], bf16, tag="xs")
                nc.sync.dma_start(out=xs[:], in_=xsT[:, esl])
                xt = iop.tile([128, EPG], bf16, tag="xt")
                nc.sync.dma_start(out=xt[:], in_=xtT[:, esl])
                sg = iop.tile([128, EPG], bf16, tag="sg")
                nc.sync.dma_start(out=sg[:], in_=S2[:, esl])
                qsb = wp.tile([128, CPG * 256], bf16, tag="qsb")
                kvsb = wp.tile([128, CPG * 512], bf16, tag="kvsb")
                for i in range(CPG):
                    ei = slice(i * 128, (i + 1) * 128)
                    ps_q = pp.tile([128, 256], f32, tag="psq")
                    ps_kv = pp.tile([128, 512], f32, tag="pskv")
                    # bias rows pre-loaded via ones-row matmul, then the
                    # projection accumulates on top (start=False)
                    nc.tensor.matmul(ps_q[:], lhsT=ones[:, :],
                                     rhs=brow[:, 0:256], start=True, stop=False)
                    nc.tensor.matmul(ps_kv[:], lhsT=ones[:, :],
                                     rhs=brow[:, 256:768], start=True, stop=False)
                    nc.tensor.matmul(ps_q[:], lhsT=xs[:, ei],
                                     rhs=wt[:, 0:256], start=False, stop=True)
                    nc.tensor.matmul(ps_kv[:], lhsT=xt[:, ei],
                                     rhs=wt[:, 256:768], start=False, stop=True)
                    nc.scalar.activation(out=qsb[:, i * 256:(i + 1) * 256],
                                         in_=ps_q[:], func=Copy)
                    nc.scalar.activation(out=kvsb[:, i * 512:(i + 1) * 512],
                                         in_=ps_kv[:], func=Copy)
                return xs, xt, sg, qsb, kvsb

            def dve_group(g, tiles):
                xs, xt, sg, qsb, kvsb = tiles
                sgrp = wp.tile([128, CPG * 64], f32, tag="sgrp")
                prodg = wp.tile([128, CPG * 2048], bf16, tag="prod")
                for i in range(CPG):
                    ci = slice(i * 256, (i + 1) * 256)
                    # prod[e, h, g, d] = Q[e,h,d] * K[e,g,d]
                    qa = (qsb[:, ci]
                          .rearrange("p (h d) -> p h d", h=H)
                          .unsqueeze(2).to_broadcast([128, H, H, D]))
                    ka = (kvsb[:, i * 512:i * 512 + 256]
                          .rearrange("p (g d) -> p g d", g=H)
                          .unsqueeze(1).to_broadcast([128, H, H, D]))
                    nc.vector.tensor_tensor(
                        out=prodg[:, i * 2048:(i + 1) * 2048]
                            .rearrange("p (h g d) -> p h g d", h=H, g=H),
                        in0=qa, in1=ka, op=MUL)

                # group-wide log-tree reduce over d (TT-adds run 2/cycle vs
                # 1 for tensor_reduce; one instruction per level for all 6
                # chunks amortizes the SBUF access bubble)
                A = CPG * 64
                pr = prodg[:].rearrange("p (a d) -> p a d", d=32)
                t1 = wp.tile([128, A * 16], bf16, tag="t1")
                t1r = t1[:].rearrange("p (a d) -> p a d", d=16)
                nc.vector.tensor_tensor(out=t1r, in0=pr[:, :, 0:16],
                                        in1=pr[:, :, 16:32], op=ADD)
                t2 = wp.tile([128, A * 8], bf16, tag="t2")
                t2r = t2[:].rearrange("p (a d) -> p a d", d=8)
                nc.vector.tensor_tensor(out=t2r, in0=t1r[:, :, 0:8],
                                        in1=t1r[:, :, 8:16], op=ADD)
                t3 = wp.tile([128, A * 4], bf16, tag="t3")
                t3r = t3[:].rearrange("p (a d) -> p a d", d=4)
                nc.vector.tensor_tensor(out=t3r, in0=t2r[:, :, 0:4],
                                        in1=t2r[:, :, 4:8], op=ADD)
                t4 = wp.tile([128, A * 2], bf16, tag="t4")
                t4r = t4[:].rearrange("p (a d) -> p a d", d=2)
                nc.vector.tensor_tensor(out=t4r, in0=t3r[:, :, 0:2],
                                        in1=t3r[:, :, 2:4], op=ADD)
                nc.vector.tensor_tensor(
                    out=sgrp[:].rearrange("p (a o) -> p a o", o=1),
                    in0=t4r[:, :, 0:1], in1=t4r[:, :, 1:2], op=ADD)

                # softmax over g for the whole group
                u = wp.tile([128, CPG * 64], f32, tag="u")
                nc.scalar.activation(out=u[:], in_=sgrp[:], func=Exp,
                                     scale=float(1.0 / np.sqrt(D)))
                zs = wp.tile([128, CPG * 8], f32, tag="zs")
                nc.vector.tensor_reduce(
                    out=zs[:], in_=u[:].rearrange("p (a g) -> p a g", g=H),
                    axis=X, op=ADD)
                rinv = wp.tile([128, CPG * 8], f32, tag="rinv")
                nc.vector.reciprocal(out=rinv[:], in_=zs[:])
                attn = wp.tile([128, CPG * 64], bf16, tag="attn")
                nc.vector.tensor_tensor(
                    out=attn[:].rearrange("p (a g) -> p a g", g=H),
                    in0=u[:].rearrange("p (a g) -> p a g", g=H),
                    in1=rinv[:].rearrange("p (a o) -> p a o", o=1)
                        .to_broadcast([128, CPG * 8, H]),
                    op=MUL)

                prod2g = wp.tile([128, CPG * 2048], bf16, tag="prod")
                for i in range(CPG):
                    # msg[e, h, d] = sum_g attn[e,h,g] * V[e,d,g] (V host-permuted)
                    aa = (attn[:, i * 64:(i + 1) * 64]
                          .rearrange("p (h g) -> p h g", h=H)
                          .unsqueeze(2).to_broadcast([128, H, D, H]))
                    va = (kvsb[:, i * 512 + 256:(i + 1) * 512]
                          .rearrange("p (d g) -> p d g", d=D)
                          .unsqueeze(1).to_broadcast([128, H, D, H]))
                    nc.vector.tensor_tensor(
                        out=prod2g[:, i * 2048:(i + 1) * 2048]
                            .rearrange("p (h d g) -> p h d g", h=H, d=D),
                        in0=aa, in1=va, op=MUL)
                # group-wide tree over g
                B = CPG * 256
                p2 = prod2g[:].rearrange("p (a g) -> p a g", g=8)
                r1 = wp.tile([128, B * 4], bf16, tag="t1")
                r1r = r1[:].rearrange("p (a g) -> p a g", g=4)
                nc.vector.tensor_tensor(out=r1r, in0=p2[:, :, 0:4],
                                        in1=p2[:, :, 4:8], op=ADD)
                r2 = wp.tile([128, B * 2], bf16, tag="t2")
                r2r = r2[:].rearrange("p (a g) -> p a g", g=2)
                nc.vector.tensor_tensor(out=r2r, in0=r1r[:, :, 0:2],
                                        in1=r1r[:, :, 2:4], op=ADD)

                # segment-sum matmuls absorb the final tree level: the two
                # pair halves of r2 are fed as strided lhsT slices and the
                # g-sum completes in the f32 PSUM accumulation
                agg1 = pa.tile([128, 128], f32, tag="agg1")
                agg2 = pa.tile([128, 128], f32, tag="agg2")
                for i in range(CPG):
                    ei = slice(i * 128, (i + 1) * 128)
                    a0 = i * 256
                    for j in (0, 1):
                        st = (i == 0 and j == 0)
                        sp = (i == CPG - 1 and j == 1)
                        nc.tensor.matmul(agg1[:],
                                         lhsT=r2r[:, a0:a0 + 128, j:j + 1],
                                         rhs=sg[:, ei], start=st, stop=sp)
                        nc.tensor.matmul(agg2[:],
                                         lhsT=r2r[:, a0 + 128:a0 + 256, j:j + 1],
                                         rhs=sg[:, ei], start=st, stop=sp)

                a1 = wp.tile([128, 128], bf16, tag="a1")
                nc.scalar.activation(out=a1[:], in_=agg1[:], func=Copy)
                a2 = wp.tile([128, 128], bf16, tag="a2")
                nc.scalar.activation(out=a2[:], in_=agg2[:], func=Copy)
                mt = pa.tile([128, 128], f32, tag="mt")
                nc.tensor.matmul(mt[:], lhsT=w2[:, 0:128], rhs=a1[:],
                                 start=True, stop=False)
                nc.tensor.matmul(mt[:], lhsT=w2[:, 128:256], rhs=a2[:],
                                 start=False, stop=True)
                nc.scalar.activation(out=outsb[:, g * 128:(g + 1) * 128],
                                     in_=mt[:], func=Copy)

            pending = proj_group(0)
            for g in range(G):
                nxt = proj_group(g + 1) if g + 1 < G else None
                dve_group(g, pending)
                pending = nxt

            nc.sync.dma_start(out=out[:, :], in_=outsb[:])
    return nc


def _pack_cores(x, src, tgt, et):
    """Group-pack each core's edges: <=128 distinct src nodes and <=EPG
    edges per group. Returns per-core packing plus global G."""
    cores = []
    Gmax = 1
    for c in range(N_CORES):
        idx = np.nonzero(src // NPC == c)[0]
        s_loc = src[idx] - c * NPC
        order = np.argsort(s_loc, kind="stable")
        idx, s_loc = idx[order], s_loc[order]
        deg = np.bincount(s_loc, minlength=NPC)
        nz = np.nonzero(deg)[0]
        # sequential node-granularity packing
        node_grp = np.empty(len(nz), np.int64)
        node_slot = np.empty(len(nz), np.int64)
        g = 0; nslots = 0; nedges = 0
        for j, n in enumerate(nz):
            d = deg[n]
            if nslots >= 128 or nedges + d > EPG:
                g += 1; nslots = 0; nedges = 0
            node_grp[j] = g
            node_slot[j] = nslots
            nslots += 1
            nedges += d
        G_core = g + 1
        Gmax = max(Gmax, G_core)
        cores.append((idx, s_loc, deg, nz, node_grp, node_slot, G_core))
    return cores, Gmax


def prepare(node_features, edges, edge_types, Wq, bq, Wk, bk, Wv, bv,
            edge_emb, Wo, bo):
    """Host-side sharding/pack. Returns (nc, in_maps, meta)."""
    x = np.asarray(node_features, dtype=np.float32)
    edges = np.asarray(edges, dtype=np.int64)
    et = np.asarray(edge_types, dtype=np.int64)
    Wq = np.asarray(Wq, np.float32); bq = np.asarray(bq, np.float32)
    Wk = np.asarray(Wk, np.float32); bk = np.asarray(bk, np.float32)
    Wv = np.asarray(Wv, np.float32); bv = np.asarray(bv, np.float32)
    edge_emb = np.asarray(edge_emb, np.float32)
    Wo = np.asarray(Wo, np.float32); bo = np.asarray(bo, np.float32)

    src, tgt = edges[:, 0], edges[:, 1]
    cores, G = _pack_cores(x, src, tgt, et)
    E_pad = G * EPG

    # host-side: edge-type embedding + output bias terms (linear in counts)
    M = edge_emb @ Wo.T                                  # [3, 128]
    cnt = np.zeros((N_NODES, edge_emb.shape[0]), np.float32)
    np.add.at(cnt, (src, et), 1.0)
    deg_all = np.bincount(src, minlength=N_NODES).astype(np.float32)
    cnt_extra = cnt @ M + deg_all[:, None] * bo[None, :]  # [N, 128]

    # shared weight blocks
    Wvp = Wv.reshape(H, D, IN_DIM).transpose(1, 0, 2).reshape(HID, IN_DIM)
    bvp = bv.reshape(H, D).T.reshape(HID)
    Wt_np = np.concatenate([Wq.T, Wk.T, Wvp.T], axis=1).astype(BF16)
    Brow_np = np.concatenate([bq, bk, bvp])[None, :].astype(BF16)
    Ones_np = np.ones((1, 128), BF16)
    WoT = Wo.T.astype(np.float32)
    W2_np = np.concatenate([WoT[0:128], WoT[128:256]], axis=1).astype(BF16)

    in_maps = []
    slot_nodes = []
    for c in range(N_CORES):
        idx, s_loc, deg, nz, node_grp, node_slot, G_core = cores[c]
        # per-node group/slot -> per-edge positions (edges are sorted by
        # s_loc so each node's edges are consecutive)
        grp_of_node = np.zeros(NPC, np.int64)
        slot_of_node = np.zeros(NPC, np.int64)
        grp_of_node[nz] = node_grp
        slot_of_node[nz] = node_slot
        # edge offset within its group: running count per group
        e_grp = grp_of_node[s_loc]
        # within-group position: stable running index per group value
        grp_counts = np.bincount(e_grp, minlength=G)
        grp_starts = np.zeros(G, np.int64)
        grp_starts[1:] = np.cumsum(grp_counts)[:-1]
        # edges sorted by s_loc are also sorted by group (groups ascend)
        within = np.arange(len(idx)) - grp_starts[e_grp]
        pos = e_grp * EPG + within

        xs_full = np.zeros((E_pad, IN_DIM), np.float32)
        xs_full[pos] = x[src[idx]]
        xt_full = np.zeros((E_pad, IN_DIM), np.float32)
        xt_full[pos] = x[tgt[idx]]
        S_full = np.zeros((E_pad, 128), np.float32)
        S_full[pos, slot_of_node[s_loc]] = 1.0

        T = G * CPG
        in_maps.append({
            "xsT": np.ascontiguousarray(xs_full.T).astype(BF16),
            "xtT": np.ascontiguousarray(xt_full.T).astype(BF16),
            "S2": np.ascontiguousarray(
                S_full.reshape(T, 128, 128).transpose(1, 0, 2)
                .reshape(128, T * 128)).astype(BF16),
            "Wt": Wt_np, "Brow": Brow_np, "Ones": Ones_np, "W2": W2_np,
        })
        node_of_slot = np.full(G * 128, -1, np.int64)
        node_of_slot[node_grp * 128 + node_slot] = nz + c * NPC
        slot_nodes.append(node_of_slot)

    if G not in _prog_cache:
        nc = _build_program(G)
        nc.finalize()
        _prog_cache[G] = nc
    nc = _prog_cache[G]
    meta = {"slot_nodes": slot_nodes, "cnt_extra": cnt_extra, "G": G}
    return nc, in_maps, meta


def unpack(outs, meta):
    messages = meta["cnt_extra"].astype(np.float32).copy()
    for c in range(N_CORES):
        o = np.asarray(outs[c]["out"], dtype=np.float32)  # [128, G*128]
        nos = meta["slot_nodes"][c]
        valid = nos >= 0
        messages[nos[valid]] += o[:, valid].T
    return messages


def kernel(node_features, edges, edge_types, Wq, bq, Wk, bk, Wv, bv,
           edge_emb, Wo, bo):
    nc, in_maps, meta = prepare(node_features, edges, edge_types, Wq, bq,
                                Wk, bk, Wv, bv, edge_emb, Wo, bo)
    res = run_bass_kernel_spmd(nc, in_maps, core_ids=list(range(N_CORES)))
    return unpack(res.results, meta)


# revision 11
# speedup vs baseline: 1.1099x; 1.0504x over previous
"""GNN message-passing kernel for Trainium2 (8 NeuronCores, edge-parallel).

Strategy: shard edges by source-node range. Each core packs its edges into
"groups" of <=128 distinct src nodes and <=768 edges (6 chunks of 128).
Each group owns 128 output slots; host maps slots back to node ids.
Outputs are disjoint -> no collective; host concatenates + adds the
edge-type-embedding/bias terms (computed host-side from counts).

Device pipeline per chunk (128 edges):
  PE : Q/K/V projections with bias pre-loaded into PSUM via a ones-row
       matmul (start=True), so no bias-add is needed on DVE.
  ACT: PSUM->SBUF bf16 eviction copies; exp(scores) per group.
  DVE: 8x8x32 attention via broadcast-AP multiply + log-tree adds
       (tensor_reduce runs at 1 elem/cycle; TT-adds run at 2/cycle).
  PE : segment-sum via one-hot matmul (S from host), final Wo projection.
"""

import sys

sys.path.insert(0, "/opt/trn_rl_repo")

import numpy as np
import ml_dtypes

from concourse import bass, bacc, mybir
import concourse.tile as tile
from concourse.bass_utils import run_bass_kernel_spmd

N_NODES = 50000
N_CORES = 8
NPC = N_NODES // N_CORES  # 6250
IN_DIM = 128
HID = 256
H = 8
D = 32
CPG = 6                   # chunks per group
EPG = CPG * 128           # 768 edges per group

BF16 = ml_dtypes.bfloat16
_prog_cache = {}


def _build_program(G):
    T = G * CPG
    E_pad = T * 128
    f32, bf16 = mybir.dt.float32, mybir.dt.bfloat16
    X = mybir.AxisListType.X
    MUL, ADD = mybir.AluOpType.mult, mybir.AluOpType.add
    Copy = mybir.ActivationFunctionType.Copy
    Exp = mybir.ActivationFunctionType.Exp

    nc = bacc.Bacc("TRN2", target_bir_lowering=False)
    xsT = nc.dram_tensor("xsT", [128, E_pad], bf16, kind="ExternalInput")
    xtT = nc.dram_tensor("xtT", [128, E_pad], bf16, kind="ExternalInput")
    S2 = nc.dram_tensor("S2", [128, E_pad], bf16, kind="ExternalInput")
    Wt = nc.dram_tensor("Wt", [128, 768], bf16, kind="ExternalInput")
    Brow = nc.dram_tensor("Brow", [1, 768], bf16, kind="ExternalInput")
    Ones = nc.dram_tensor("Ones", [1, 128], bf16, kind="ExternalInput")
    W2 = nc.dram_tensor("W2", [128, 256], bf16, kind="ExternalInput")
    out = nc.dram_tensor("out", [128, G * 128], f32, kind="ExternalOutput")

    with tile.TileContext(nc) as tc:
        with tc.tile_pool(name="const", bufs=1) as cp, \
             tc.tile_pool(name="io", bufs=2) as iop, \
             tc.tile_pool(name="work", bufs=2) as wp, \
             tc.tile_pool(name="pproj", bufs=2, space="PSUM") as pp, \
             tc.tile_pool(name="pacc", bufs=1, space="PSUM") as pa:

            wt = cp.tile([128, 768], bf16)
            nc.sync.dma_start(out=wt[:], in_=Wt[:, :])
            brow = cp.tile([1, 768], bf16)
            nc.sync.dma_start(out=brow[:], in_=Brow[:, :])
            ones = cp.tile([1, 128], bf16)
            nc.sync.dma_start(out=ones[:], in_=Ones[:, :])
            w2 = cp.tile([128, 256], bf16)
            nc.sync.dma_start(out=w2[:], in_=W2[:, :])
            outsb = cp.tile([128, G * 128], f32)

            def proj_group(g):
                """DMA + QKV projections + evictions for group g. Emitted a
                group ahead so ACT's eviction copies are queued before the
                previous group's exp and never stall the DVE."""
                esl = slice(g * EPG, (g + 1) * EPG)
                xs = iop.tile([128, EPG], bf16, tag="xs")
                nc.sync.dma_start(out=xs[:], in_=xsT[:, esl])
                xt = iop.tile([128, EPG], bf16, tag="xt")
                nc.sync.dma_start(out=xt[:], in_=xtT[:, esl])
                sg = iop.tile([128, EPG], bf16, tag="sg")
                nc.sync.dma_start(out=sg[:], in_=S2[:, esl])
                qsb = wp.tile([128, CPG * 256], bf16, tag="qsb")
                kvsb = wp.tile([128, CPG * 512], bf16, tag="kvsb")
                for i in range(CPG):
                    ei = slice(i * 128, (i + 1) * 128)
                    ps_q = pp.tile([128, 256], f32, tag="psq")
                    ps_kv = pp.tile([128, 512], f32, tag="pskv")
                    # bias rows pre-loaded via ones-row matmul, then the
                    # projection accumulates on top (start=False)
                    nc.tensor.matmul(ps_q[:], lhsT=ones[:, :],
                                     rhs=brow[:, 0:256], start=True, stop=False)
                    nc.tensor.matmul(ps_kv[:], lhsT=ones[:, :],
                                     rhs=brow[:, 256:768], start=True, stop=False)
                    nc.tensor.matmul(ps_q[:], lhsT=xs[:, ei],
                                     rhs=wt[:, 0:256], start=False, stop=True)
                    nc.tensor.matmul(ps_kv[:], lhsT=xt[:, ei],
                                     rhs=wt[:, 256:768], start=False, stop=True)
                    nc.scalar.activation(out=qsb[:, i * 256:(i + 1) * 256],
                                         in_=ps_q[:], func=Copy)
                    nc.scalar.activation(out=kvsb[:, i * 512:(i + 1) * 512],
                                         in_=ps_kv[:], func=Copy)
                return xs, xt, sg, qsb, kvsb

            def emit_prods(tiles):
                xs, xt, sg, qsb, kvsb = tiles
                prodg = wp.tile([128, CPG * 2048], bf16, tag="prod")
                for i in range(CPG):
                    ci = slice(i * 256, (i + 1) * 256)
                    # prod[e, h, g, d] = Q[e,h,d] * K[e,g,d]
                    qa = (qsb[:, ci]
                          .rearrange("p (h d) -> p h d", h=H)
                          .unsqueeze(2).to_broadcast([128, H, H, D]))
                    ka = (kvsb[:, i * 512:i * 512 + 256]
                          .rearrange("p (g d) -> p g d", g=H)
                          .unsqueeze(1).to_broadcast([128, H, H, D]))
                    nc.vector.tensor_tensor(
                        out=prodg[:, i * 2048:(i + 1) * 2048]
                            .rearrange("p (h g d) -> p h g d", h=H, g=H),
                        in0=qa, in1=ka, op=MUL)
                return prodg

            def dve_group(g, tiles, prodg, hoist):
                """hoist() is called between the scores tree (whose tail
                feeds ACT's exp) and the softmax tail that consumes exp's
                output, so the DVE computes the next group's products
                instead of stalling on the cross-engine exp latency."""
                xs, xt, sg, qsb, kvsb = tiles
                sgrp = wp.tile([128, CPG * 64], f32, tag="sgrp")

                # group-wide log-tree reduce over d (TT-adds run 2/cycle vs
                # 1 for tensor_reduce; one instruction per level for all 6
                # chunks amortizes the SBUF access bubble)
                A = CPG * 64
                pr = prodg[:].rearrange("p (a d) -> p a d", d=32)
                t1 = wp.tile([128, A * 16], bf16, tag="t1")
                t1r = t1[:].rearrange("p (a d) -> p a d", d=16)
                nc.vector.tensor_tensor(out=t1r, in0=pr[:, :, 0:16],
                                        in1=pr[:, :, 16:32], op=ADD)
                t2 = wp.tile([128, A * 8], bf16, tag="t2")
                t2r = t2[:].rearrange("p (a d) -> p a d", d=8)
                nc.vector.tensor_tensor(out=t2r, in0=t1r[:, :, 0:8],
                                        in1=t1r[:, :, 8:16], op=ADD)
                t3 = wp.tile([128, A * 4], bf16, tag="t3")
                t3r = t3[:].rearrange("p (a d) -> p a d", d=4)
                nc.vector.tensor_tensor(out=t3r, in0=t2r[:, :, 0:4],
                                        in1=t2r[:, :, 4:8], op=ADD)
                t4 = wp.tile([128, A * 2], bf16, tag="t4")
                t4r = t4[:].rearrange("p (a d) -> p a d", d=2)
                nc.vector.tensor_tensor(out=t4r, in0=t3r[:, :, 0:2],
                                        in1=t3r[:, :, 2:4], op=ADD)
                nc.vector.tensor_tensor(
                    out=sgrp[:].rearrange("p (a o) -> p a o", o=1),
                    in0=t4r[:, :, 0:1], in1=t4r[:, :, 1:2], op=ADD)

                # softmax over g for the whole group
                u = wp.tile([128, CPG * 64], f32, tag="u")
                nc.scalar.activation(out=u[:], in_=sgrp[:], func=Exp,
                                     scale=float(1.0 / np.sqrt(D)))
                nxt_prod = hoist()
                zs = wp.tile([128, CPG * 8], f32, tag="zs")
                nc.vector.tensor_reduce(
                    out=zs[:], in_=u[:].rearrange("p (a g) -> p a g", g=H),
                    axis=X, op=ADD)
                rinv = wp.tile([128, CPG * 8], f32, tag="rinv")
                nc.vector.reciprocal(out=rinv[:], in_=zs[:])
                attn = wp.tile([128, CPG * 64], bf16, tag="attn")
                nc.vector.tensor_tensor(
                    out=attn[:].rearrange("p (a g) -> p a g", g=H),
                    in0=u[:].rearrange("p (a g) -> p a g", g=H),
                    in1=rinv[:].rearrange("p (a o) -> p a o", o=1)
                        .to_broadcast([128, CPG * 8, H]),
                    op=MUL)

                prod2g = wp.tile([128, CPG * 2048], bf16, tag="prod")
                for i in range(CPG):
                    # msg[e, h, d] = sum_g attn[e,h,g] * V[e,d,g] (V host-permuted)
                    aa = (attn[:, i * 64:(i + 1) * 64]
                          .rearrange("p (h g) -> p h g", h=H)
                          .unsqueeze(2).to_broadcast([128, H, D, H]))
                    va = (kvsb[:, i * 512 + 256:(i + 1) * 512]
                          .rearrange("p (d g) -> p d g", d=D)
                          .unsqueeze(1).to_broadcast([128, H, D, H]))
                    nc.vector.tensor_tensor(
                        out=prod2g[:, i * 2048:(i + 1) * 2048]
                            .rearrange("p (h d g) -> p h d g", h=H, d=D),
                        in0=aa, in1=va, op=MUL)
                # group-wide tree over g
                B = CPG * 256
                p2 = prod2g[:].rearrange("p (a g) -> p a g", g=8)
                r1 = wp.tile([128, B * 4], bf16, tag="t1")
                r1r = r1[:].rearrange("p (a g) -> p a g", g=4)
                nc.vector.tensor_tensor(out=r1r, in0=p2[:, :, 0:4],
                                        in1=p2[:, :, 4:8], op=ADD)
                r2 = wp.tile([128, B * 2], bf16, tag="t2")
                r2r = r2[:].rearrange("p (a g) -> p a g", g=2)
                nc.vector.tensor_tensor(out=r2r, in0=r1r[:, :, 0:2],
                                        in1=r1r[:, :, 2:4], op=ADD)

                # segment-sum matmuls absorb the final tree level: the two
                # pair halves of r2 are fed as strided lhsT slices and the
                # g-sum completes in the f32 PSUM accumulation
                agg1 = pa.tile([128, 128], f32, tag="agg1")
                agg2 = pa.tile([128, 128], f32, tag="agg2")
                for i in range(CPG):
                    ei = slice(i * 128, (i + 1) * 128)
                    a0 = i * 256
                    for j in (0, 1):
                        st = (i == 0 and j == 0)
                        sp = (i == CPG - 1 and j == 1)
                        nc.tensor.matmul(agg1[:],
                                         lhsT=r2r[:, a0:a0 + 128, j:j + 1],
                                         rhs=sg[:, ei], start=st, stop=sp)
                        nc.tensor.matmul(agg2[:],
                                         lhsT=r2r[:, a0 + 128:a0 + 256, j:j + 1],
                                         rhs=sg[:, ei], start=st, stop=sp)

                a1 = wp.tile([128, 128], bf16, tag="a1")
                nc.scalar.activation(out=a1[:], in_=agg1[:], func=Copy)
                a2 = wp.tile([128, 128], bf16, tag="a2")
                nc.scalar.activation(out=a2[:], in_=agg2[:], func=Copy)
                mt = pa.tile([128, 128], f32, tag="mt")
                nc.tensor.matmul(mt[:], lhsT=w2[:, 0:128], rhs=a1[:],
                                 start=True, stop=False)
                nc.tensor.matmul(mt[:], lhsT=w2[:, 128:256], rhs=a2[:],
                                 start=False, stop=True)
                nc.scalar.activation(out=outsb[:, g * 128:(g + 1) * 128],
                                     in_=mt[:], func=Copy)
                return nxt_prod

            pending = proj_group(0)
            pend_prod = emit_prods(pending)
            for g in range(G):
                nxt = proj_group(g + 1) if g + 1 < G else None
                hoist = (lambda t=nxt: emit_prods(t)) if nxt else (lambda: None)
                nxt_prod = dve_group(g, pending, pend_prod, hoist)
                pending, pend_prod = nxt, nxt_prod

            nc.sync.dma_start(out=out[:, :], in_=outsb[:])
    return nc


def _pack_cores(x, src, tgt, et):
    """Group-pack each core's edges: <=128 distinct src nodes and <=EPG
    edges per group. Returns per-core packing plus global G."""
    cores = []
    Gmax = 1
    for c in range(N_CORES):
        idx = np.nonzero(src // NPC == c)[0]
        s_loc = src[idx] - c * NPC
        order = np.argsort(s_loc, kind="stable")
        idx, s_loc = idx[order], s_loc[order]
        deg = np.bincount(s_loc, minlength=NPC)
        nz = np.nonzero(deg)[0]
        # sequential node-granularity packing
        node_grp = np.empty(len(nz), np.int64)
        node_slot = np.empty(len(nz), np.int64)
        g = 0; nslots = 0; nedges = 0
        for j, n in enumerate(nz):
            d = deg[n]
            if nslots >= 128 or nedges + d > EPG:
                g += 1; nslots = 0; nedges = 0
            node_grp[j] = g
            node_slot[j] = nslots
            nslots += 1
            nedges += d
        G_core = g + 1
        Gmax = max(Gmax, G_core)
        cores.append((idx, s_loc, deg, nz, node_grp, node_slot, G_core))
    return cores, Gmax


def prepare(node_features, edges, edge_types, Wq, bq, Wk, bk, Wv, bv,
            edge_emb, Wo, bo):
    """Host-side sharding/pack. Returns (nc, in_maps, meta)."""
    x = np.asarray(node_features, dtype=np.float32)
    edges = np.asarray(edges, dtype=np.int64)
    et = np.asarray(edge_types, dtype=np.int64)
    Wq = np.asarray(Wq, np.float32); bq = np.asarray(bq, np.float32)
    Wk = np.asarray(Wk, np.float32); bk = np.asarray(bk, np.float32)
    Wv = np.asarray(Wv, np.float32); bv = np.asarray(bv, np.float32)
    edge_emb = np.asarray(edge_emb, np.float32)
    Wo = np.asarray(Wo, np.float32); bo = np.asarray(bo, np.float32)

    src, tgt = edges[:, 0], edges[:, 1]
    cores, G = _pack_cores(x, src, tgt, et)
    E_pad = G * EPG

    # host-side: edge-type embedding + output bias terms (linear in counts)
    M = edge_emb @ Wo.T                                  # [3, 128]
    cnt = np.zeros((N_NODES, edge_emb.shape[0]), np.float32)
    np.add.at(cnt, (src, et), 1.0)
    deg_all = np.bincount(src, minlength=N_NODES).astype(np.float32)
    cnt_extra = cnt @ M + deg_all[:, None] * bo[None, :]  # [N, 128]

    # shared weight blocks
    Wvp = Wv.reshape(H, D, IN_DIM).transpose(1, 0, 2).reshape(HID, IN_DIM)
    bvp = bv.reshape(H, D).T.reshape(HID)
    Wt_np = np.concatenate([Wq.T, Wk.T, Wvp.T], axis=1).astype(BF16)
    Brow_np = np.concatenate([bq, bk, bvp])[None, :].astype(BF16)
    Ones_np = np.ones((1, 128), BF16)
    WoT = Wo.T.astype(np.float32)
    W2_np = np.concatenate([WoT[0:128], WoT[128:256]], axis=1).astype(BF16)

    in_maps = []
    slot_nodes = []
    for c in range(N_CORES):
        idx, s_loc, deg, nz, node_grp, node_slot, G_core = cores[c]
        # per-node group/slot -> per-edge positions (edges are sorted by
        # s_loc so each node's edges are consecutive)
        grp_of_node = np.zeros(NPC, np.int64)
        slot_of_node = np.zeros(NPC, np.int64)
        grp_of_node[nz] = node_grp
        slot_of_node[nz] = node_slot
        # edge offset within its group: running count per group
        e_grp = grp_of_node[s_loc]
        # within-group position: stable running index per group value
        grp_counts = np.bincount(e_grp, minlength=G)
        grp_starts = np.zeros(G, np.int64)
        grp_starts[1:] = np.cumsum(grp_counts)[:-1]
        # edges sorted by s_loc are also sorted by group (groups ascend)
        within = np.arange(len(idx)) - grp_starts[e_grp]
        pos = e_grp * EPG + within

        xs_full = np.zeros((E_pad, IN_DIM), np.float32)
        xs_full[pos] = x[src[idx]]
        xt_full = np.zeros((E_pad, IN_DIM), np.float32)
        xt_full[pos] = x[tgt[idx]]
        S_full = np.zeros((E_pad, 128), np.float32)
        S_full[pos, slot_of_node[s_loc]] = 1.0

        T = G * CPG
        in_maps.append({
            "xsT": np.ascontiguousarray(xs_full.T).astype(BF16),
            "xtT": np.ascontiguousarray(xt_full.T).astype(BF16),
            "S2": np.ascontiguousarray(
                S_full.reshape(T, 128, 128).transpose(1, 0, 2)
                .reshape(128, T * 128)).astype(BF16),
            "Wt": Wt_np, "Brow": Brow_np, "Ones": Ones_np, "W2": W2_np,
        })
        node_of_slot = np.full(G * 128, -1, np.int64)
        node_of_slot[node_grp * 128 + node_slot] = nz + c * NPC
        slot_nodes.append(node_of_slot)

    if G not in _prog_cache:
        nc = _build_program(G)
        nc.finalize()
        _prog_cache[G] = nc
    nc = _prog_cache[G]
    meta = {"slot_nodes": slot_nodes, "cnt_extra": cnt_extra, "G": G}
    return nc, in_maps, meta


def unpack(outs, meta):
    messages = meta["cnt_extra"].astype(np.float32).copy()
    for c in range(N_CORES):
        o = np.asarray(outs[c]["out"], dtype=np.float32)  # [128, G*128]
        nos = meta["slot_nodes"][c]
        valid = nos >= 0
        messages[nos[valid]] += o[:, valid].T
    return messages


def kernel(node_features, edges, edge_types, Wq, bq, Wk, bk, Wv, bv,
           edge_emb, Wo, bo):
    nc, in_maps, meta = prepare(node_features, edges, edge_types, Wq, bq,
                                Wk, bk, Wv, bv, edge_emb, Wo, bo)
    res = run_bass_kernel_spmd(nc, in_maps, core_ids=list(range(N_CORES)))
    return unpack(res.results, meta)


# revision 12
# speedup vs baseline: 1.1293x; 1.0175x over previous
"""GNN message-passing kernel for Trainium2 (8 NeuronCores, edge-parallel).

Strategy: shard edges by source-node range. Each core packs its edges into
"groups" of <=128 distinct src nodes and <=768 edges (6 chunks of 128).
Each group owns 128 output slots; host maps slots back to node ids.
Outputs are disjoint -> no collective; host concatenates + adds the
edge-type-embedding/bias terms (computed host-side from counts).

Device pipeline per chunk (128 edges):
  PE : Q/K/V projections with bias pre-loaded into PSUM via a ones-row
       matmul (start=True), so no bias-add is needed on DVE.
  ACT: PSUM->SBUF bf16 eviction copies; exp(scores) per group.
  DVE: 8x8x32 attention via broadcast-AP multiply + log-tree adds
       (tensor_reduce runs at 1 elem/cycle; TT-adds run at 2/cycle).
  PE : segment-sum via one-hot matmul (S from host), final Wo projection.
"""

import sys

sys.path.insert(0, "/opt/trn_rl_repo")

import numpy as np
import ml_dtypes

from concourse import bass, bacc, mybir
import concourse.tile as tile
from concourse.bass_utils import run_bass_kernel_spmd

N_NODES = 50000
N_CORES = 8
NPC = N_NODES // N_CORES  # 6250
IN_DIM = 128
HID = 256
H = 8
D = 32
CPG = 6                   # chunks per group
EPG = CPG * 128           # 768 edges per group

BF16 = ml_dtypes.bfloat16
_prog_cache = {}


def _build_program(G):
    T = G * CPG
    E_pad = T * 128
    f32, bf16 = mybir.dt.float32, mybir.dt.bfloat16
    X = mybir.AxisListType.X
    MUL, ADD = mybir.AluOpType.mult, mybir.AluOpType.add
    Copy = mybir.ActivationFunctionType.Copy
    Exp = mybir.ActivationFunctionType.Exp

    nc = bacc.Bacc("TRN2", target_bir_lowering=False)
    xsT = nc.dram_tensor("xsT", [128, E_pad], bf16, kind="ExternalInput")
    xtT = nc.dram_tensor("xtT", [128, E_pad], bf16, kind="ExternalInput")
    S2 = nc.dram_tensor("S2", [128, E_pad], bf16, kind="ExternalInput")
    Wt = nc.dram_tensor("Wt", [128, 768], bf16, kind="ExternalInput")
    Brow = nc.dram_tensor("Brow", [1, 768], bf16, kind="ExternalInput")
    Ones = nc.dram_tensor("Ones", [1, 128], bf16, kind="ExternalInput")
    W2 = nc.dram_tensor("W2", [128, 256], bf16, kind="ExternalInput")
    out = nc.dram_tensor("out", [128, G * 128], f32, kind="ExternalOutput")

    with tile.TileContext(nc) as tc:
        with tc.tile_pool(name="const", bufs=1) as cp, \
             tc.tile_pool(name="io", bufs=2) as iop, \
             tc.tile_pool(name="work", bufs=2) as wp, \
             tc.tile_pool(name="pproj", bufs=2, space="PSUM") as pp, \
             tc.tile_pool(name="pacc", bufs=1, space="PSUM") as pa:

            wt = cp.tile([128, 768], bf16)
            nc.sync.dma_start(out=wt[:], in_=Wt[:, :])
            brow = cp.tile([1, 768], bf16)
            nc.sync.dma_start(out=brow[:], in_=Brow[:, :])
            ones = cp.tile([1, 128], bf16)
            nc.sync.dma_start(out=ones[:], in_=Ones[:, :])
            w2 = cp.tile([128, 256], bf16)
            nc.sync.dma_start(out=w2[:], in_=W2[:, :])
            outsb = cp.tile([128, G * 128], f32)

            def proj_group(g):
                """DMA + QKV projections + evictions for group g. Emitted a
                group ahead so ACT's eviction copies are queued before the
                previous group's exp and never stall the DVE."""
                esl = slice(g * EPG, (g + 1) * EPG)
                xs = iop.tile([128, EPG], bf16, tag="xs")
                nc.sync.dma_start(out=xs[:], in_=xsT[:, esl])
                xt = iop.tile([128, EPG], bf16, tag="xt")
                nc.sync.dma_start(out=xt[:], in_=xtT[:, esl])
                sg = iop.tile([128, EPG], bf16, tag="sg")
                nc.sync.dma_start(out=sg[:], in_=S2[:, esl])
                qsb = wp.tile([128, CPG * 256], bf16, tag="qsb")
                kvsb = wp.tile([128, CPG * 512], bf16, tag="kvsb")
                for i in range(CPG):
                    ei = slice(i * 128, (i + 1) * 128)
                    ps_q = pp.tile([128, 256], f32, tag="psq")
                    ps_kv = pp.tile([128, 512], f32, tag="pskv")
                    # bias rows pre-loaded via ones-row matmul, then the
                    # projection accumulates on top (start=False)
                    nc.tensor.matmul(ps_q[:], lhsT=ones[:, :],
                                     rhs=brow[:, 0:256], start=True, stop=False)
                    nc.tensor.matmul(ps_kv[:], lhsT=ones[:, :],
                                     rhs=brow[:, 256:768], start=True, stop=False)
                    nc.tensor.matmul(ps_q[:], lhsT=xs[:, ei],
                                     rhs=wt[:, 0:256], start=False, stop=True)
                    nc.tensor.matmul(ps_kv[:], lhsT=xt[:, ei],
                                     rhs=wt[:, 256:768], start=False, stop=True)
                    nc.scalar.activation(out=qsb[:, i * 256:(i + 1) * 256],
                                         in_=ps_q[:], func=Copy)
                    nc.scalar.activation(out=kvsb[:, i * 512:(i + 1) * 512],
                                         in_=ps_kv[:], func=Copy)
                return xs, xt, sg, qsb, kvsb

            def emit_prods(tiles, prodg=None, lo=0, hi=CPG):
                xs, xt, sg, qsb, kvsb = tiles
                if prodg is None:
                    prodg = wp.tile([128, CPG * 2048], bf16, tag="prod")
                for i in range(lo, hi):
                    ci = slice(i * 256, (i + 1) * 256)
                    # prod[e, h, g, d] = Q[e,h,d] * K[e,g,d]
                    qa = (qsb[:, ci]
                          .rearrange("p (h d) -> p h d", h=H)
                          .unsqueeze(2).to_broadcast([128, H, H, D]))
                    ka = (kvsb[:, i * 512:i * 512 + 256]
                          .rearrange("p (g d) -> p g d", g=H)
                          .unsqueeze(1).to_broadcast([128, H, H, D]))
                    nc.vector.tensor_tensor(
                        out=prodg[:, i * 2048:(i + 1) * 2048]
                            .rearrange("p (h g d) -> p h g d", h=H, g=H),
                        in0=qa, in1=ka, op=MUL)
                return prodg

            def dve_group(g, tiles, prodg, hoist):
                """hoist() is called between the scores tree (whose tail
                feeds ACT's exp) and the softmax tail that consumes exp's
                output, so the DVE computes the next group's products
                instead of stalling on the cross-engine exp latency."""
                xs, xt, sg, qsb, kvsb = tiles
                sgrp = wp.tile([128, CPG * 64], f32, tag="sgrp")

                # group-wide log-tree reduce over d (TT-adds run 2/cycle vs
                # 1 for tensor_reduce; one instruction per level for all 6
                # chunks amortizes the SBUF access bubble)
                A = CPG * 64
                pr = prodg[:].rearrange("p (a d) -> p a d", d=32)
                t1 = wp.tile([128, A * 16], bf16, tag="t1")
                t1r = t1[:].rearrange("p (a d) -> p a d", d=16)
                nc.vector.tensor_tensor(out=t1r, in0=pr[:, :, 0:16],
                                        in1=pr[:, :, 16:32], op=ADD)
                t2 = wp.tile([128, A * 8], bf16, tag="t2")
                t2r = t2[:].rearrange("p (a d) -> p a d", d=8)
                nc.vector.tensor_tensor(out=t2r, in0=t1r[:, :, 0:8],
                                        in1=t1r[:, :, 8:16], op=ADD)
                t3 = wp.tile([128, A * 4], bf16, tag="t3")
                t3r = t3[:].rearrange("p (a d) -> p a d", d=4)
                nc.vector.tensor_tensor(out=t3r, in0=t2r[:, :, 0:4],
                                        in1=t2r[:, :, 4:8], op=ADD)
                t4 = wp.tile([128, A * 2], bf16, tag="t4")
                t4r = t4[:].rearrange("p (a d) -> p a d", d=2)
                nc.vector.tensor_tensor(out=t4r, in0=t3r[:, :, 0:2],
                                        in1=t3r[:, :, 2:4], op=ADD)
                nc.vector.tensor_tensor(
                    out=sgrp[:].rearrange("p (a o) -> p a o", o=1),
                    in0=t4r[:, :, 0:1], in1=t4r[:, :, 1:2], op=ADD)

                # softmax over g for the whole group (u in bf16 so the
                # normalize TT can run in 2x mode; the reciprocal is expanded
                # to a dense [p, 384] tile on ACT since a stride-0 broadcast
                # operand would force the TT back to 1x)
                u = wp.tile([128, CPG * 64], bf16, tag="u")
                nc.scalar.activation(out=u[:], in_=sgrp[:], func=Exp,
                                     scale=float(1.0 / np.sqrt(D)))
                nxt_prod = hoist(None, 0, 3)
                zs = wp.tile([128, CPG * 8], f32, tag="zs")
                nc.vector.tensor_reduce(
                    out=zs[:], in_=u[:].rearrange("p (a g) -> p a g", g=H),
                    axis=X, op=ADD)
                rinv = wp.tile([128, CPG * 8], f32, tag="rinv")
                nc.vector.reciprocal(out=rinv[:], in_=zs[:])
                rexp = wp.tile([128, CPG * 64], bf16, tag="rexp")
                nc.scalar.activation(
                    out=rexp[:].rearrange("p (a g) -> p a g", g=H),
                    in_=rinv[:].rearrange("p (a o) -> p a o", o=1)
                        .to_broadcast([128, CPG * 8, H]),
                    func=Copy)
                nxt_prod = hoist(nxt_prod, 3, CPG)
                attn = wp.tile([128, CPG * 64], bf16, tag="attn")
                nc.vector.tensor_tensor(
                    out=attn[:].rearrange("p (a g) -> p a g", g=H),
                    in0=u[:].rearrange("p (a g) -> p a g", g=H),
                    in1=rexp[:].rearrange("p (a g) -> p a g", g=H),
                    op=MUL)

                prod2g = wp.tile([128, CPG * 2048], bf16, tag="prod")
                for i in range(CPG):
                    # msg[e, h, d] = sum_g attn[e,h,g] * V[e,d,g] (V host-permuted)
                    aa = (attn[:, i * 64:(i + 1) * 64]
                          .rearrange("p (h g) -> p h g", h=H)
                          .unsqueeze(2).to_broadcast([128, H, D, H]))
                    va = (kvsb[:, i * 512 + 256:(i + 1) * 512]
                          .rearrange("p (d g) -> p d g", d=D)
                          .unsqueeze(1).to_broadcast([128, H, D, H]))
                    nc.vector.tensor_tensor(
                        out=prod2g[:, i * 2048:(i + 1) * 2048]
                            .rearrange("p (h d g) -> p h d g", h=H, d=D),
                        in0=aa, in1=va, op=MUL)
                # group-wide tree over g
                B = CPG * 256
                p2 = prod2g[:].rearrange("p (a g) -> p a g", g=8)
                r1 = wp.tile([128, B * 4], bf16, tag="t1")
                r1r = r1[:].rearrange("p (a g) -> p a g", g=4)
                nc.vector.tensor_tensor(out=r1r, in0=p2[:, :, 0:4],
                                        in1=p2[:, :, 4:8], op=ADD)
                r2 = wp.tile([128, B * 2], bf16, tag="t2")
                r2r = r2[:].rearrange("p (a g) -> p a g", g=2)
                nc.vector.tensor_tensor(out=r2r, in0=r1r[:, :, 0:2],
                                        in1=r1r[:, :, 2:4], op=ADD)

                # segment-sum matmuls absorb the final tree level: the two
                # pair halves of r2 are fed as strided lhsT slices and the
                # g-sum completes in the f32 PSUM accumulation
                agg1 = pa.tile([128, 128], f32, tag="agg1")
                agg2 = pa.tile([128, 128], f32, tag="agg2")
                for i in range(CPG):
                    ei = slice(i * 128, (i + 1) * 128)
                    a0 = i * 256
                    for j in (0, 1):
                        st = (i == 0 and j == 0)
                        sp = (i == CPG - 1 and j == 1)
                        nc.tensor.matmul(agg1[:],
                                         lhsT=r2r[:, a0:a0 + 128, j:j + 1],
                                         rhs=sg[:, ei], start=st, stop=sp)
                        nc.tensor.matmul(agg2[:],
                                         lhsT=r2r[:, a0 + 128:a0 + 256, j:j + 1],
                                         rhs=sg[:, ei], start=st, stop=sp)

                a1 = wp.tile([128, 128], bf16, tag="a1")
                nc.scalar.activation(out=a1[:], in_=agg1[:], func=Copy)
                a2 = wp.tile([128, 128], bf16, tag="a2")
                nc.scalar.activation(out=a2[:], in_=agg2[:], func=Copy)
                mt = pa.tile([128, 128], f32, tag="mt")
                nc.tensor.matmul(mt[:], lhsT=w2[:, 0:128], rhs=a1[:],
                                 start=True, stop=False)
                nc.tensor.matmul(mt[:], lhsT=w2[:, 128:256], rhs=a2[:],
                                 start=False, stop=True)
                nc.scalar.activation(out=outsb[:, g * 128:(g + 1) * 128],
                                     in_=mt[:], func=Copy)
                return nxt_prod

            pending = proj_group(0)
            pend_prod = emit_prods(pending)
            for g in range(G):
                nxt = proj_group(g + 1) if g + 1 < G else None
                hoist = ((lambda pg, lo, hi, t=nxt: emit_prods(t, pg, lo, hi))
                         if nxt else (lambda pg, lo, hi: None))
                nxt_prod = dve_group(g, pending, pend_prod, hoist)
                pending, pend_prod = nxt, nxt_prod

            nc.sync.dma_start(out=out[:, :], in_=outsb[:])
    return nc


def _pack_cores(x, src, tgt, et):
    """Group-pack each core's edges: <=128 distinct src nodes and <=EPG
    edges per group. Returns per-core packing plus global G."""
    cores = []
    Gmax = 1
    for c in range(N_CORES):
        idx = np.nonzero(src // NPC == c)[0]
        s_loc = src[idx] - c * NPC
        order = np.argsort(s_loc, kind="stable")
        idx, s_loc = idx[order], s_loc[order]
        deg = np.bincount(s_loc, minlength=NPC)
        nz = np.nonzero(deg)[0]
        # sequential node-granularity packing
        node_grp = np.empty(len(nz), np.int64)
        node_slot = np.empty(len(nz), np.int64)
        g = 0; nslots = 0; nedges = 0
        for j, n in enumerate(nz):
            d = deg[n]
            if nslots >= 128 or nedges + d > EPG:
                g += 1; nslots = 0; nedges = 0
            node_grp[j] = g
            node_slot[j] = nslots
            nslots += 1
            nedges += d
        G_core = g + 1
        Gmax = max(Gmax, G_core)
        cores.append((idx, s_loc, deg, nz, node_grp, node_slot, G_core))
    return cores, Gmax


def prepare(node_features, edges, edge_types, Wq, bq, Wk, bk, Wv, bv,
            edge_emb, Wo, bo):
    """Host-side sharding/pack. Returns (nc, in_maps, meta)."""
    x = np.asarray(node_features, dtype=np.float32)
    edges = np.asarray(edges, dtype=np.int64)
    et = np.asarray(edge_types, dtype=np.int64)
    Wq = np.asarray(Wq, np.float32); bq = np.asarray(bq, np.float32)
    Wk = np.asarray(Wk, np.float32); bk = np.asarray(bk, np.float32)
    Wv = np.asarray(Wv, np.float32); bv = np.asarray(bv, np.float32)
    edge_emb = np.asarray(edge_emb, np.float32)
    Wo = np.asarray(Wo, np.float32); bo = np.asarray(bo, np.float32)

    src, tgt = edges[:, 0], edges[:, 1]
    cores, G = _pack_cores(x, src, tgt, et)
    E_pad = G * EPG

    # host-side: edge-type embedding + output bias terms (linear in counts)
    M = edge_emb @ Wo.T                                  # [3, 128]
    cnt = np.zeros((N_NODES, edge_emb.shape[0]), np.float32)
    np.add.at(cnt, (src, et), 1.0)
    deg_all = np.bincount(src, minlength=N_NODES).astype(np.float32)
    cnt_extra = cnt @ M + deg_all[:, None] * bo[None, :]  # [N, 128]

    # shared weight blocks
    Wvp = Wv.reshape(H, D, IN_DIM).transpose(1, 0, 2).reshape(HID, IN_DIM)
    bvp = bv.reshape(H, D).T.reshape(HID)
    Wt_np = np.concatenate([Wq.T, Wk.T, Wvp.T], axis=1).astype(BF16)
    Brow_np = np.concatenate([bq, bk, bvp])[None, :].astype(BF16)
    Ones_np = np.ones((1, 128), BF16)
    WoT = Wo.T.astype(np.float32)
    W2_np = np.concatenate([WoT[0:128], WoT[128:256]], axis=1).astype(BF16)

    in_maps = []
    slot_nodes = []
    for c in range(N_CORES):
        idx, s_loc, deg, nz, node_grp, node_slot, G_core = cores[c]
        # per-node group/slot -> per-edge positions (edges are sorted by
        # s_loc so each node's edges are consecutive)
        grp_of_node = np.zeros(NPC, np.int64)
        slot_of_node = np.zeros(NPC, np.int64)
        grp_of_node[nz] = node_grp
        slot_of_node[nz] = node_slot
        # edge offset within its group: running count per group
        e_grp = grp_of_node[s_loc]
        # within-group position: stable running index per group value
        grp_counts = np.bincount(e_grp, minlength=G)
        grp_starts = np.zeros(G, np.int64)
        grp_starts[1:] = np.cumsum(grp_counts)[:-1]
        # edges sorted by s_loc are also sorted by group (groups ascend)
        within = np.arange(len(idx)) - grp_starts[e_grp]
        pos = e_grp * EPG + within

        xs_full = np.zeros((E_pad, IN_DIM), np.float32)
        xs_full[pos] = x[src[idx]]
        xt_full = np.zeros((E_pad, IN_DIM), np.float32)
        xt_full[pos] = x[tgt[idx]]
        S_full = np.zeros((E_pad, 128), np.float32)
        S_full[pos, slot_of_node[s_loc]] = 1.0

        T = G * CPG
        in_maps.append({
            "xsT": np.ascontiguousarray(xs_full.T).astype(BF16),
            "xtT": np.ascontiguousarray(xt_full.T).astype(BF16),
            "S2": np.ascontiguousarray(
                S_full.reshape(T, 128, 128).transpose(1, 0, 2)
                .reshape(128, T * 128)).astype(BF16),
            "Wt": Wt_np, "Brow": Brow_np, "Ones": Ones_np, "W2": W2_np,
        })
        node_of_slot = np.full(G * 128, -1, np.int64)
        node_of_slot[node_grp * 128 + node_slot] = nz + c * NPC
        slot_nodes.append(node_of_slot)

    if G not in _prog_cache:
        nc = _build_program(G)
        nc.finalize()
        _prog_cache[G] = nc
    nc = _prog_cache[G]
    meta = {"slot_nodes": slot_nodes, "cnt_extra": cnt_extra, "G": G}
    return nc, in_maps, meta


def unpack(outs, meta):
    messages = meta["cnt_extra"].astype(np.float32).copy()
    for c in range(N_CORES):
        o = np.asarray(outs[c]["out"], dtype=np.float32)  # [128, G*128]
        nos = meta["slot_nodes"][c]
        valid = nos >= 0
        messages[nos[valid]] += o[:, valid].T
    return messages


def kernel(node_features, edges, edge_types, Wq, bq, Wk, bk, Wv, bv,
           edge_emb, Wo, bo):
    nc, in_maps, meta = prepare(node_features, edges, edge_types, Wq, bq,
                                Wk, bk, Wv, bv, edge_emb, Wo, bo)
    res = run_bass_kernel_spmd(nc, in_maps, core_ids=list(range(N_CORES)))
    return unpack(res.results, meta)
